# revision 5
# baseline (speedup 1.0000x reference)
"""TRN2 Bass kernel: fused attention block (QKV proj + RoPE + causal SDPA + O proj).

Sharding: 8 cores = 2 (batch) x 4 (head groups of 4 heads).  Each core computes a
partial o_proj for its batch; host sums the 4 partials per batch.

All matmuls run in float32r (TF32-like, full PE rate at N>=256; measured
resid_var ~2e-8 vs fp64 for a 128-deep dot product).

Dataflow is fully transposed: hidden^T [H,S] streams through QKV matmuls to
produce Q^T,K^T [HD,S] (roped) and V [S,HD]; attention computes
scores^T = K^T.T @ Q^T per 128k x 512q block, exp on ScalarE (softmax max-trick
skipped: logits are ~N(0,1), bounded), PV as V.T-free accumulation
out^T = V.T @ P.T, denominator via ones-vector matmul, normalization by
GPSIMD partition-broadcast reciprocal.  o_proj: out^T = wo_slice @ attn^T.
"""

import math
import numpy as np

B, S, H = 2, 2048, 2048
NH, HD = 16, 128
P = 128
NHPC = 4                  # heads per core
HDPC = NHPC * HD          # 512
KT = H // P               # 16 contraction tiles
QBLK = 512
KBLK = 128
NQT = S // QBLK           # 4
NKB = S // KBLK           # 16
NSUB = S // P             # 16
GW = 1024                 # phase-1 s-group width
NG = S // GW              # 2
MAXPAT = 16

_prog_cache = {}


def _classify_mask(mask2d):
    """Per (qt, kb) block: 'skip' (fully masked), 'plain' (zero), or pattern id.

    Patterns are the transposed [KBLK, QBLK] additive-mask blocks, deduped.
    """
    pats = {}
    pat_list = []
    btypes = []
    for qt in range(NQT):
        row = []
        for kb in range(NKB):
            blk = mask2d[qt * QBLK:(qt + 1) * QBLK, kb * KBLK:(kb + 1) * KBLK]
            if np.all(blk == 0.0):
                row.append(("plain", -1))
            elif np.all(blk <= -1e4):
                row.append(("skip", -1))
            else:
                tb = np.ascontiguousarray(blk.T.astype(np.float32))
                key = tb.tobytes()
                if key not in pats:
                    pats[key] = len(pat_list)
                    pat_list.append(tb)
                row.append(("pat", pats[key]))
        btypes.append(row)
    assert len(pat_list) <= MAXPAT, f"too many mask patterns: {len(pat_list)}"
    for row in btypes:
        assert any(t != "skip" for t, _ in row), "fully-masked query tile"
    return btypes, pat_list


def _build_program(btypes, n_pat):
    import concourse.bacc as bacc
    import concourse.tile as tile
    import concourse.mybir as mybir

    dt = mybir.dt
    f32, f32r = dt.float32, dt.float32r
    AF = mybir.ActivationFunctionType

    nc = bacc.Bacc(None, target_bir_lowering=False)

    hT = nc.declare_dram_parameter("hT", [H, S], f32r, isOutput=False)
    wq = nc.declare_dram_parameter("wq", [H, HDPC], f32r, isOutput=False)
    wk = nc.declare_dram_parameter("wk", [H, HDPC], f32r, isOutput=False)
    wv = nc.declare_dram_parameter("wv", [H, HDPC], f32r, isOutput=False)
    wo = nc.declare_dram_parameter("wo", [HDPC, H], f32r, isOutput=False)
    cosq = nc.declare_dram_parameter("cosq", [P, S], f32, isOutput=False)
    sinq = nc.declare_dram_parameter("sinq", [P, S], f32, isOutput=False)
    cosk = nc.declare_dram_parameter("cosk", [P, S], f32, isOutput=False)
    sink = nc.declare_dram_parameter("sink", [P, S], f32, isOutput=False)
    mpat = nc.declare_dram_parameter("mpat", [max(n_pat, 1), P, QBLK], f32,
                                     isOutput=False)
    outp = nc.declare_dram_parameter("outp", [H, S], f32r, isOutput=True)

    qsp = nc.dram_tensor("qspill", [NHPC, P, S], f32r)
    ksp = nc.dram_tensor("kspill", [NHPC, P, S], f32r)
    vsp = nc.dram_tensor("vspill", [P, NSUB, HDPC], f32r)

    NCH = GW // QBLK  # 512-chunks per group

    with tile.TileContext(nc) as tc:
        with tc.tile_pool(name="res", bufs=1) as res:
            attn = res.tile([P, NHPC * S], f32r, tag="attn")
            ones_f = res.tile([P, 1], f32, tag="ones_f")
            nc.gpsimd.memset(ones_f[:], 1.0)
            ones = res.tile([P, 1], f32r, tag="ones")
            nc.vector.tensor_copy(ones[:], ones_f[:])

            # ---------------- Phase 1: QKV projection + RoPE ----------------
            for g in range(NG):
                gc = g * GW  # global col base of this group

                with tc.tile_pool(name=f"hid{g}", bufs=1) as hpool:
                    hid = hpool.tile([P, KT * GW], f32r, tag="hid")
                    hidv = hid[:].rearrange("p (k s) -> p k s", k=KT)
                    hsrc = hT[:, gc:gc + GW].rearrange("(k p) s -> p k s", p=P)
                    for c in range(2):
                        nc.sync.dma_start(hidv[:, c * 8:(c + 1) * 8, :],
                                          hsrc[:, c * 8:(c + 1) * 8, :])

                    # ---- QK passes ----
                    with tc.tile_pool(name=f"qk{g}", bufs=2) as wpool, \
                         tc.tile_pool(name=f"tb{g}", bufs=1) as tpool, \
                         tc.tile_pool(name=f"st{g}", bufs=6) as spool, \
                         tc.tile_pool(name=f"tm{g}", bufs=2) as mpool, \
                         tc.tile_pool(name=f"ps{g}", bufs=8, space="PSUM") as pp:

                        tabs = {}
                        for nm, src in (("cq", cosq), ("sq", sinq),
                                        ("ck", cosk), ("sk", sink)):
                            t = tpool.tile([P, GW], f32, tag=nm)
                            nc.sync.dma_start(t[:], src[:, gc:gc + GW])
                            tabs[nm] = t

                        def rope_evac(ps, cost, sint, dst):
                            t1 = mpool.tile([64, QBLK], f32, tag="t1")
                            t2 = mpool.tile([64, QBLK], f32, tag="t2")
                            t3 = mpool.tile([64, QBLK], f32, tag="t3")
                            t4 = mpool.tile([64, QBLK], f32, tag="t4")
                            nc.vector.tensor_mul(t1[:], ps[0:64, :], cost[0:64, :])
                            nc.vector.tensor_mul(t2[:], ps[64:128, :], sint[0:64, :])
                            nc.vector.tensor_sub(dst[0:64, :], t1[:], t2[:])
                            nc.vector.tensor_mul(t3[:], ps[64:128, :], cost[64:128, :])
                            nc.vector.tensor_mul(t4[:], ps[0:64, :], sint[64:128, :])
                            nc.vector.tensor_add(dst[64:128, :], t3[:], t4[:])

                        for wdram, h0, isq in ((wq, 0, True), (wq, 2, True),
                                               (wk, 0, False), (wk, 2, False)):
                            wt = wpool.tile([P, KT * 256], f32r, tag="w")
                            wtv = wt[:].rearrange("p (k m) -> p k m", k=KT)
                            wsrc = wdram[:, h0 * HD:(h0 + 2) * HD]
                            nc.sync.dma_start(
                                wtv, wsrc.rearrange("(k p) m -> p k m", p=P))
                            for m in range(2):  # head within pair
                                h = h0 + m
                                pss = [pp.tile([P, QBLK], f32, tag="ps1", name=f"ps1_{h}_{ch}")
                                       for ch in range(NCH)]
                                for kt in range(KT):
                                    for ch in range(NCH):
                                        nc.tensor.matmul(
                                            pss[ch][:],
                                            wtv[:, kt, m * HD:(m + 1) * HD],
                                            hidv[:, kt, ch * QBLK:(ch + 1) * QBLK],
                                            start=(kt == 0), stop=(kt == KT - 1))
                                for ch in range(NCH):
                                    stg = spool.tile([P, QBLK], f32r, tag="stg")
                                    cs = slice(ch * QBLK, (ch + 1) * QBLK)
                                    if isq:
                                        rope_evac(pss[ch], tabs["cq"][:, cs],
                                                  tabs["sq"][:, cs], stg)
                                        nc.sync.dma_start(
                                            qsp[h, :, gc + ch * QBLK:
                                                gc + (ch + 1) * QBLK], stg[:])
                                    else:
                                        rope_evac(pss[ch], tabs["ck"][:, cs],
                                                  tabs["sk"][:, cs], stg)
                                        nc.sync.dma_start(
                                            ksp[h, :, gc + ch * QBLK:
                                                gc + (ch + 1) * QBLK], stg[:])

                    # ---- V passes ----
                    with tc.tile_pool(name=f"vw{g}", bufs=1) as vwpool, \
                         tc.tile_pool(name=f"vs{g}", bufs=6) as vspool, \
                         tc.tile_pool(name=f"vp{g}", bufs=8, space="PSUM") as vpp:
                        wvt = vwpool.tile([P, KT * HDPC], f32r, tag="wv")
                        wvv = wvt[:].rearrange("p (k m) -> p k m", k=KT)
                        nc.sync.dma_start(
                            wvv, wv[:].rearrange("(k p) m -> p k m", p=P))
                        for half in range(2):
                            pss = [vpp.tile([P, HDPC], f32, tag="psv", name=f"psv_{half}_{sl}")
                                   for sl in range(4)]
                            for kt in range(KT):
                                for sl in range(4):
                                    sub_l = half * 4 + sl
                                    nc.tensor.matmul(
                                        pss[sl][:],
                                        hidv[:, kt, sub_l * P:(sub_l + 1) * P],
                                        wvv[:, kt, :],
                                        start=(kt == 0), stop=(kt == KT - 1))
                            for sl in range(4):
                                sub = g * (GW // P) + half * 4 + sl
                                stg = vspool.tile([P, HDPC], f32r, tag="vst")
                                nc.scalar.copy(stg[:], pss[sl][:])
                                nc.sync.dma_start(vsp[:, sub, :], stg[:])

            # ---------------- Phase 2: attention ----------------
            with tc.tile_pool(name="kv2", bufs=2) as kv2, \
                 tc.tile_pool(name="q2", bufs=3) as q2, \
                 tc.tile_pool(name="ex2", bufs=4) as ex2, \
                 tc.tile_pool(name="ms2", bufs=1) as ms2, \
                 tc.tile_pool(name="sm2", bufs=3) as sm2, \
                 tc.tile_pool(name="psS", bufs=3, space="PSUM") as psS, \
                 tc.tile_pool(name="psO", bufs=2, space="PSUM") as psO, \
                 tc.tile_pool(name="psL", bufs=2, space="PSUM") as psL:

                mp = ms2.tile([P, max(n_pat, 1) * QBLK], f32, tag="mp")
                nc.sync.dma_start(
                    mp[:].rearrange("p (n q) -> p n q", q=QBLK),
                    mpat[:].rearrange("n p q -> p n q"))

                for h in range(NHPC):
                    kh = kv2.tile([P, S], f32r, tag="kh")
                    nc.sync.dma_start(kh[:], ksp[h])
                    vh = kv2.tile([P, S], f32r, tag="vh")
                    nc.sync.dma_start(
                        vh[:].rearrange("p (s d) -> p s d", d=HD),
                        vsp[:, :, h * HD:(h + 1) * HD])
                    for qt in range(NQT):
                        qtl = q2.tile([P, QBLK], f32r, tag="q")
                        nc.sync.dma_start(
                            qtl[:], qsp[h, :, qt * QBLK:(qt + 1) * QBLK])
                        blocks = [kb for kb in range(NKB)
                                  if btypes[qt][kb][0] != "skip"]
                        po = psO.tile([P, QBLK], f32, tag="po")
                        pl = psL.tile([1, QBLK], f32, tag="pl")
                        for i, kb in enumerate(blocks):
                            first, last = (i == 0), (i == len(blocks) - 1)
                            ps = psS.tile([P, QBLK], f32, tag="ps")
                            nc.tensor.matmul(
                                ps[:], kh[:, kb * KBLK:(kb + 1) * KBLK], qtl[:],
                                start=True, stop=True)
                            typ, pid = btypes[qt][kb]
                            if typ == "pat":
                                nc.vector.tensor_add(
                                    ps[:], ps[:],
                                    mp[:, pid * QBLK:(pid + 1) * QBLK])
                            ex = ex2.tile([P, QBLK], f32r, tag="ex")
                            nc.scalar.activation(ex[:], ps[:], AF.Exp)
                            nc.tensor.matmul(
                                po[:], vh[:, kb * HD:(kb + 1) * HD], ex[:],
                                start=first, stop=last)
                            nc.tensor.matmul(
                                pl[:], ones[:], ex[:],
                                start=first, stop=last)
                        lr = sm2.tile([1, QBLK], f32, tag="lr")
                        nc.vector.reciprocal(lr[:], pl[:])
                        lb = sm2.tile([P, QBLK], f32, tag="lb")
                        nc.gpsimd.partition_broadcast(lb[:], lr[:])
                        nc.vector.tensor_mul(
                            attn[:, h * S + qt * QBLK:h * S + (qt + 1) * QBLK],
                            po[:], lb[:])

            # ---------------- Phase 3: output projection (partial) ----------
            with tc.tile_pool(name="wo3", bufs=1) as wo3, \
                 tc.tile_pool(name="ot3", bufs=3) as ot3, \
                 tc.tile_pool(name="psC", bufs=8, space="PSUM") as psC:
                wos = wo3.tile([P, NHPC * H], f32r, tag="wos")
                nc.sync.dma_start(
                    wos[:].rearrange("p (k m) -> p k m", k=NHPC),
                    wo[:].rearrange("(k p) m -> p k m", p=P))
                for mb in range(H // P):
                    pcs = [psC.tile([P, QBLK], f32, tag="pc", name=f"pc_{mb}_{st3}") for st3 in range(4)]
                    for hk in range(NHPC):
                        for st3 in range(4):
                            nc.tensor.matmul(
                                pcs[st3][:],
                                wos[:, hk * H + mb * P:hk * H + (mb + 1) * P],
                                attn[:, hk * S + st3 * QBLK:
                                     hk * S + (st3 + 1) * QBLK],
                                start=(hk == 0), stop=(hk == NHPC - 1))
                    ot = ot3.tile([P, S], f32r, tag="ot")
                    for st3 in range(4):
                        nc.scalar.copy(ot[:, st3 * QBLK:(st3 + 1) * QBLK],
                                       pcs[st3][:])
                    nc.sync.dma_start(outp[mb * P:(mb + 1) * P, :], ot[:])

    nc.finalize()
    return nc


def _get_program(mask2d):
    key = hash(mask2d.tobytes())
    if key not in _prog_cache:
        btypes, pat_list = _classify_mask(mask2d)
        nc = _build_program(btypes, len(pat_list))
        _prog_cache[key] = (nc, btypes, pat_list)
    return _prog_cache[key]


def kernel(hidden_states, rope_cos, rope_sin, attention_mask, w_qkv, w_o):
    from concourse.bass_utils import run_bass_kernel_spmd

    hidden_states = np.asarray(hidden_states, dtype=np.float32)
    rope_cos = np.asarray(rope_cos, dtype=np.float32)
    rope_sin = np.asarray(rope_sin, dtype=np.float32)
    attention_mask = np.asarray(attention_mask, dtype=np.float32)
    w_qkv = np.asarray(w_qkv, dtype=np.float32)
    w_o = np.asarray(w_o, dtype=np.float32)

    mask2d = np.ascontiguousarray(attention_mask.reshape(S, S))
    nc, btypes, pat_list = _get_program(mask2d)
    n_pat = len(pat_list)
    mpat = (np.stack(pat_list) if n_pat
            else np.zeros((1, P, QBLK), np.float32))

    scale = 1.0 / math.sqrt(HD)
    cosT = np.ascontiguousarray(rope_cos.T)          # [HD, S]
    sinT = np.ascontiguousarray(rope_sin.T)
    cosq = np.ascontiguousarray(cosT * scale)
    sinq = np.ascontiguousarray(sinT * scale)

    hTs = [np.ascontiguousarray(hidden_states[b].T) for b in range(B)]

    in_maps = []
    for c in range(8):
        b, hg = c // 4, c % 4
        r0 = hg * HDPC
        in_maps.append({
            "hT": hTs[b],
            "wq": np.ascontiguousarray(w_qkv[r0:r0 + HDPC, :].T),
            "wk": np.ascontiguousarray(w_qkv[H + r0:H + r0 + HDPC, :].T),
            "wv": np.ascontiguousarray(w_qkv[2 * H + r0:2 * H + r0 + HDPC, :].T),
            "wo": np.ascontiguousarray(w_o[:, r0:r0 + HDPC].T),
            "cosq": cosq, "sinq": sinq, "cosk": cosT, "sink": sinT,
            "mpat": mpat,
        })

    import os
    kw = {}
    if os.environ.get("BASS_KERNEL_TRACE"):
        kw["trace"] = True
    res = run_bass_kernel_spmd(nc, in_maps, list(range(8)), **kw)
    global LAST_RESULTS
    LAST_RESULTS = res

    out = np.empty((B, S, H), dtype=np.float32)
    for b in range(B):
        acc = np.zeros((H, S), dtype=np.float64)
        for hg in range(4):
            acc += res.results[b * 4 + hg]["outp"].astype(np.float64)
        out[b] = acc.T.astype(np.float32)
    return out


# revision 9
# speedup vs baseline: 1.0423x; 1.0423x over previous
"""TRN2 Bass kernel: fused attention block (QKV proj + RoPE + causal SDPA + O proj).

Sharding: 8 cores = 2 (batch) x 4 (head groups of 4 heads).  Each core computes a
partial o_proj for its batch; host sums the 4 partials per batch.

All matmuls run in float32r (TF32-like, full PE rate at N>=256; measured
resid_var ~2e-8 vs fp64 for a 128-deep dot product).

Dataflow is fully transposed: hidden^T [H,S] streams through QKV matmuls to
produce Q^T,K^T [HD,S] (roped) and V [S,HD]; attention computes
scores^T = K^T.T @ Q^T per 128k x 512q block, exp on ScalarE (softmax max-trick
skipped: logits are ~N(0,1), bounded), PV as V.T-free accumulation
out^T = V.T @ P.T, denominator via ones-vector matmul, normalization by
GPSIMD partition-broadcast reciprocal.  o_proj: out^T = wo_slice @ attn^T.
"""

import math
import numpy as np

B, S, H = 2, 2048, 2048
NH, HD = 16, 128
P = 128
NHPC = 4                  # heads per core
HDPC = NHPC * HD          # 512
KT = H // P               # 16 contraction tiles
QBLK = 512
KBLK = 128
NQT = S // QBLK           # 4
NKB = S // KBLK           # 16
NSUB = S // P             # 16
GW = 1024                 # phase-1 s-group width
NG = S // GW              # 2
MAXPAT = 16

_prog_cache = {}


def _classify_mask(mask2d):
    """Per (qt, kb) block: 'skip' (fully masked), 'plain' (zero), or pattern id.

    Patterns are the transposed [KBLK, QBLK] additive-mask blocks, deduped.
    """
    pats = {}
    pat_list = []
    btypes = []
    for qt in range(NQT):
        row = []
        for kb in range(NKB):
            blk = mask2d[qt * QBLK:(qt + 1) * QBLK, kb * KBLK:(kb + 1) * KBLK]
            if np.all(blk == 0.0):
                row.append(("plain", -1))
            elif np.all(blk <= -1e4):
                row.append(("skip", -1))
            else:
                tb = np.ascontiguousarray(blk.T.astype(np.float32))
                key = tb.tobytes()
                if key not in pats:
                    pats[key] = len(pat_list)
                    pat_list.append(tb)
                row.append(("pat", pats[key]))
        btypes.append(row)
    assert len(pat_list) <= MAXPAT, f"too many mask patterns: {len(pat_list)}"
    for row in btypes:
        assert any(t != "skip" for t, _ in row), "fully-masked query tile"
    return btypes, pat_list


def _build_program(btypes, n_pat):
    import concourse.bacc as bacc
    import concourse.tile as tile
    import concourse.mybir as mybir

    dt = mybir.dt
    f32, f32r = dt.float32, dt.float32r
    AF = mybir.ActivationFunctionType

    nc = bacc.Bacc(None, target_bir_lowering=False)

    hT = nc.declare_dram_parameter("hT", [H, S], f32r, isOutput=False)
    wq = nc.declare_dram_parameter("wq", [H, HDPC], f32r, isOutput=False)
    wk = nc.declare_dram_parameter("wk", [H, HDPC], f32r, isOutput=False)
    wv = nc.declare_dram_parameter("wv", [H, HDPC], f32r, isOutput=False)
    wo = nc.declare_dram_parameter("wo", [HDPC, H], f32r, isOutput=False)
    cosq = nc.declare_dram_parameter("cosq", [P, S], f32, isOutput=False)
    sinq = nc.declare_dram_parameter("sinq", [P, S], f32, isOutput=False)
    cosk = nc.declare_dram_parameter("cosk", [P, S], f32, isOutput=False)
    sink = nc.declare_dram_parameter("sink", [P, S], f32, isOutput=False)
    mpat = nc.declare_dram_parameter("mpat", [max(n_pat, 1), P, QBLK], f32,
                                     isOutput=False)
    outp = nc.declare_dram_parameter("outp", [H, S], f32r, isOutput=True)

    qsp = nc.dram_tensor("qspill", [NHPC, P, S], f32r)
    ksp = nc.dram_tensor("kspill", [NHPC, P, S], f32r)
    vsp = nc.dram_tensor("vspill", [P, NSUB, HDPC], f32r)

    NCH = GW // QBLK  # 512-chunks per group

    with tile.TileContext(nc) as tc:
        with tc.tile_pool(name="res", bufs=1) as res:
            attn = res.tile([P, NHPC * S], f32r, tag="attn")
            ones_f = res.tile([P, 1], f32, tag="ones_f")
            nc.gpsimd.memset(ones_f[:], 1.0)
            ones = res.tile([P, 1], f32r, tag="ones")
            nc.vector.tensor_copy(ones[:], ones_f[:])

            # ---------------- Phase 1: QKV projection + RoPE ----------------
            # wq/wk/wv fully resident; hidden streamed once as [128,512]
            # kt-tiles; per 512-wide s-tile: 8 QK outputs + 4 V outputs share
            # one 8-slot PSUM pool (QK evacs free slots for V).
            with tc.tile_pool(name="w1", bufs=1) as w1, \
                 tc.tile_pool(name="tb1", bufs=1) as tb1, \
                 tc.tile_pool(name="hb1", bufs=8) as hb1, \
                 tc.tile_pool(name="st1", bufs=5) as st1, \
                 tc.tile_pool(name="tm1", bufs=2) as tm1, \
                 tc.tile_pool(name="ps1", bufs=8, space="PSUM") as ps1:

                wres = {}
                for nm, wdram in (("wq", wq), ("wk", wk), ("wv", wv)):
                    wt = w1.tile([P, KT * HDPC], f32r, tag=nm, name=nm + "_sb")
                    wtv = wt[:].rearrange("p (k m) -> p k m", k=KT)
                    wsrc = wdram[:].rearrange("(k p) m -> p k m", p=P)
                    for c in range(4):
                        nc.sync.dma_start(wtv[:, c * 4:(c + 1) * 4, :],
                                          wsrc[:, c * 4:(c + 1) * 4, :])
                    wres[nm] = wtv

                tabs = {}
                for nm, src_ in (("cq", cosq), ("sq", sinq),
                                 ("ck", cosk), ("sk", sink)):
                    t = tb1.tile([P, S], f32, tag=nm, name=nm + "_sb")
                    nc.sync.dma_start(t[:], src_[:])
                    tabs[nm] = t

                def rope_evac(ps, cost, sint, dst):
                    # dst = ps*cos + rot(ps)*sin, rot = [-x2, x1] on halves
                    ta = tm1.tile([P, QBLK], f32, tag="ta")
                    tb = tm1.tile([P, QBLK], f32, tag="tb")
                    nc.vector.tensor_mul(ta[:], ps[:], cost[:])
                    nc.vector.tensor_mul(tb[0:64, :], ps[64:128, :],
                                         sint[0:64, :])
                    nc.vector.tensor_mul(tb[64:128, :], ps[0:64, :],
                                         sint[64:128, :])
                    nc.vector.tensor_add(dst[:], ta[:], tb[:])

                for st in range(S // QBLK):
                    sc = st * QBLK
                    hts = []
                    for kt in range(KT):
                        hb = hb1.tile([P, QBLK], f32r, tag="hb", bufs=8,
                                      name=f"hb_{st}_{kt}")
                        nc.sync.dma_start(
                            hb[:], hT[kt * P:(kt + 1) * P, sc:sc + QBLK])
                        hts.append(hb)

                    # 8 QK outputs: (mat, head) x 4 heads
                    qk_out = [("wq", h) for h in range(NHPC)] + \
                             [("wk", h) for h in range(NHPC)]
                    pss = [ps1.tile([P, QBLK], f32, tag="ps",
                                    name=f"ps_{st}_{oi}")
                           for oi in range(len(qk_out))]
                    for kt in range(KT):
                        for oi, (nm, h) in enumerate(qk_out):
                            nc.tensor.matmul(
                                pss[oi][:],
                                wres[nm][:, kt, h * HD:(h + 1) * HD],
                                hts[kt][:],
                                start=(kt == 0), stop=(kt == KT - 1))
                    for oi, (nm, h) in enumerate(qk_out):
                        stg = st1.tile([P, QBLK], f32r, tag="stg")
                        if nm == "wq":
                            rope_evac(pss[oi], tabs["cq"][:, sc:sc + QBLK],
                                      tabs["sq"][:, sc:sc + QBLK], stg)
                            nc.sync.dma_start(qsp[h, :, sc:sc + QBLK], stg[:])
                        else:
                            rope_evac(pss[oi], tabs["ck"][:, sc:sc + QBLK],
                                      tabs["sk"][:, sc:sc + QBLK], stg)
                            nc.sync.dma_start(ksp[h, :, sc:sc + QBLK], stg[:])

                    # 4 V outputs (s-subblocks of this s-tile); separate
                    # hidden stream so V never blocks the QK tile ring
                    psv = [ps1.tile([P, HDPC], f32, tag="ps",
                                    name=f"psv_{st}_{sl}")
                           for sl in range(4)]
                    for kt in range(KT):
                        hv = hb1.tile([P, QBLK], f32r, tag="hv", bufs=4,
                                      name=f"hv_{st}_{kt}")
                        nc.sync.dma_start(
                            hv[:], hT[kt * P:(kt + 1) * P, sc:sc + QBLK])
                        for sl in range(4):
                            nc.tensor.matmul(
                                psv[sl][:],
                                hv[:, sl * P:(sl + 1) * P],
                                wres["wv"][:, kt, :],
                                start=(kt == 0), stop=(kt == KT - 1))
                    for sl in range(4):
                        stg = st1.tile([P, HDPC], f32r, tag="stg")
                        nc.scalar.copy(stg[:], psv[sl][:])
                        nc.sync.dma_start(vsp[:, st * 4 + sl, :], stg[:])

            # ---------------- Phase 2: attention ----------------
            with tc.tile_pool(name="kv2", bufs=2) as kv2, \
                 tc.tile_pool(name="q2", bufs=3) as q2, \
                 tc.tile_pool(name="ex2", bufs=4) as ex2, \
                 tc.tile_pool(name="ms2", bufs=1) as ms2, \
                 tc.tile_pool(name="sm2", bufs=3) as sm2, \
                 tc.tile_pool(name="psS", bufs=3, space="PSUM") as psS, \
                 tc.tile_pool(name="psO", bufs=2, space="PSUM") as psO, \
                 tc.tile_pool(name="psL", bufs=2, space="PSUM") as psL:

                mp = ms2.tile([P, max(n_pat, 1) * QBLK], f32, tag="mp")
                nc.sync.dma_start(
                    mp[:].rearrange("p (n q) -> p n q", q=QBLK),
                    mpat[:].rearrange("n p q -> p n q"))

                for h in range(NHPC):
                    kh = kv2.tile([P, S], f32r, tag="kh")
                    nc.sync.dma_start(kh[:], ksp[h])
                    vh = kv2.tile([P, S], f32r, tag="vh")
                    nc.sync.dma_start(
                        vh[:].rearrange("p (s d) -> p s d", d=HD),
                        vsp[:, :, h * HD:(h + 1) * HD])
                    for qt in range(NQT):
                        qtl = q2.tile([P, QBLK], f32r, tag="q")
                        nc.sync.dma_start(
                            qtl[:], qsp[h, :, qt * QBLK:(qt + 1) * QBLK])
                        blocks = [kb for kb in range(NKB)
                                  if btypes[qt][kb][0] != "skip"]
                        po = psO.tile([P, QBLK], f32, tag="po")
                        pl = psL.tile([1, QBLK], f32, tag="pl")
                        for i, kb in enumerate(blocks):
                            first, last = (i == 0), (i == len(blocks) - 1)
                            ps = psS.tile([P, QBLK], f32, tag="ps")
                            nc.tensor.matmul(
                                ps[:], kh[:, kb * KBLK:(kb + 1) * KBLK], qtl[:],
                                start=True, stop=True)
                            typ, pid = btypes[qt][kb]
                            if typ == "pat":
                                nc.vector.tensor_add(
                                    ps[:], ps[:],
                                    mp[:, pid * QBLK:(pid + 1) * QBLK])
                            ex = ex2.tile([P, QBLK], f32r, tag="ex")
                            nc.scalar.activation(ex[:], ps[:], AF.Exp)
                            nc.tensor.matmul(
                                po[:], vh[:, kb * HD:(kb + 1) * HD], ex[:],
                                start=first, stop=last)
                            nc.tensor.matmul(
                                pl[:], ones[:], ex[:],
                                start=first, stop=last)
                        lr = sm2.tile([1, QBLK], f32, tag="lr")
                        nc.vector.reciprocal_approx_fast(lr[:], pl[:])
                        lb = sm2.tile([P, QBLK], f32, tag="lb")
                        nc.gpsimd.partition_broadcast(lb[:], lr[:])
                        nc.vector.tensor_mul(
                            attn[:, h * S + qt * QBLK:h * S + (qt + 1) * QBLK],
                            po[:], lb[:])

            # ---------------- Phase 3: output projection (partial) ----------
            with tc.tile_pool(name="wo3", bufs=1) as wo3, \
                 tc.tile_pool(name="ot3", bufs=3) as ot3, \
                 tc.tile_pool(name="psC", bufs=8, space="PSUM") as psC:
                wos = wo3.tile([P, NHPC * H], f32r, tag="wos")
                nc.sync.dma_start(
                    wos[:].rearrange("p (k m) -> p k m", k=NHPC),
                    wo[:].rearrange("(k p) m -> p k m", p=P))
                for mb in range(H // P):
                    pcs = [psC.tile([P, QBLK], f32, tag="pc", name=f"pc_{mb}_{st3}") for st3 in range(4)]
                    for hk in range(NHPC):
                        for st3 in range(4):
                            nc.tensor.matmul(
                                pcs[st3][:],
                                wos[:, hk * H + mb * P:hk * H + (mb + 1) * P],
                                attn[:, hk * S + st3 * QBLK:
                                     hk * S + (st3 + 1) * QBLK],
                                start=(hk == 0), stop=(hk == NHPC - 1))
                    ot = ot3.tile([P, S], f32r, tag="ot")
                    for st3 in range(4):
                        nc.scalar.copy(ot[:, st3 * QBLK:(st3 + 1) * QBLK],
                                       pcs[st3][:])
                    nc.sync.dma_start(outp[mb * P:(mb + 1) * P, :], ot[:])

    nc.finalize()
    return nc


def _get_program(mask2d):
    key = hash(mask2d.tobytes())
    if key not in _prog_cache:
        btypes, pat_list = _classify_mask(mask2d)
        nc = _build_program(btypes, len(pat_list))
        _prog_cache[key] = (nc, btypes, pat_list)
    return _prog_cache[key]


def kernel(hidden_states, rope_cos, rope_sin, attention_mask, w_qkv, w_o):
    from concourse.bass_utils import run_bass_kernel_spmd

    hidden_states = np.asarray(hidden_states, dtype=np.float32)
    rope_cos = np.asarray(rope_cos, dtype=np.float32)
    rope_sin = np.asarray(rope_sin, dtype=np.float32)
    attention_mask = np.asarray(attention_mask, dtype=np.float32)
    w_qkv = np.asarray(w_qkv, dtype=np.float32)
    w_o = np.asarray(w_o, dtype=np.float32)

    mask2d = np.ascontiguousarray(attention_mask.reshape(S, S))
    nc, btypes, pat_list = _get_program(mask2d)
    n_pat = len(pat_list)
    mpat = (np.stack(pat_list) if n_pat
            else np.zeros((1, P, QBLK), np.float32))

    scale = 1.0 / math.sqrt(HD)
    cosT = np.ascontiguousarray(rope_cos.T)          # [HD, S]
    sinT = rope_sin.T.copy()
    # fold the rotate-half sign into the table: out = x*cos + swap(x)*sinN
    sinT[0:64, :] *= -1.0
    sinT = np.ascontiguousarray(sinT)
    cosq = np.ascontiguousarray(cosT * scale)
    sinq = np.ascontiguousarray(sinT * scale)

    hTs = [np.ascontiguousarray(hidden_states[b].T) for b in range(B)]

    in_maps = []
    for c in range(8):
        b, hg = c // 4, c % 4
        r0 = hg * HDPC
        in_maps.append({
            "hT": hTs[b],
            "wq": np.ascontiguousarray(w_qkv[r0:r0 + HDPC, :].T),
            "wk": np.ascontiguousarray(w_qkv[H + r0:H + r0 + HDPC, :].T),
            "wv": np.ascontiguousarray(w_qkv[2 * H + r0:2 * H + r0 + HDPC, :].T),
            "wo": np.ascontiguousarray(w_o[:, r0:r0 + HDPC].T),
            "cosq": cosq, "sinq": sinq, "cosk": cosT, "sink": sinT,
            "mpat": mpat,
        })

    import os
    kw = {}
    if os.environ.get("BASS_KERNEL_TRACE"):
        kw["trace"] = True
    res = run_bass_kernel_spmd(nc, in_maps, list(range(8)), **kw)
    global LAST_RESULTS
    LAST_RESULTS = res

    out = np.empty((B, S, H), dtype=np.float32)
    for b in range(B):
        acc = np.zeros((H, S), dtype=np.float64)
        for hg in range(4):
            acc += res.results[b * 4 + hg]["outp"].astype(np.float64)
        out[b] = acc.T.astype(np.float32)
    return out


# revision 12
# speedup vs baseline: 1.1206x; 1.0751x over previous
"""TRN2 Bass kernel: fused attention block (QKV proj + RoPE + causal SDPA + O proj).

Sharding: 8 cores = 2 (batch) x 4 (head groups of 4 heads).  Each core computes a
partial o_proj for its batch; host sums the 4 partials per batch.

All matmuls run in float32r (TF32-like, full PE rate at N>=256; measured
resid_var ~2e-8 vs fp64 for a 128-deep dot product).

Dataflow is fully transposed: hidden^T [H,S] streams through QKV matmuls to
produce Q^T,K^T [HD,S] (roped) and V [S,HD]; attention computes
scores^T = K^T.T @ Q^T per 128k x 512q block, exp on ScalarE (softmax max-trick
skipped: logits are ~N(0,1), bounded), PV as V.T-free accumulation
out^T = V.T @ P.T, denominator via ones-vector matmul, normalization by
GPSIMD partition-broadcast reciprocal.  o_proj: out^T = wo_slice @ attn^T.
"""

import math
import numpy as np

B, S, H = 2, 2048, 2048
NH, HD = 16, 128
P = 128
NHPC = 4                  # heads per core
HDPC = NHPC * HD          # 512
KT = H // P               # 16 contraction tiles
QBLK = 512
KBLK = 128
NQT = S // QBLK           # 4
NKB = S // KBLK           # 16
NSUB = S // P             # 16
GW = 1024                 # phase-1 s-group width
NG = S // GW              # 2
MAXPAT = 16

_prog_cache = {}


def _classify_mask(mask2d):
    """Per (qt, kb) block: 'skip' (fully masked), 'plain' (zero), or pattern id.

    Patterns are the transposed [KBLK, QBLK] additive-mask blocks, deduped.
    """
    pats = {}
    pat_list = []
    btypes = []
    for qt in range(NQT):
        row = []
        for kb in range(NKB):
            blk = mask2d[qt * QBLK:(qt + 1) * QBLK, kb * KBLK:(kb + 1) * KBLK]
            if np.all(blk == 0.0):
                row.append(("plain", -1))
            elif np.all(blk <= -1e4):
                row.append(("skip", -1))
            else:
                tb = np.ascontiguousarray(blk.T.astype(np.float32))
                key = tb.tobytes()
                if key not in pats:
                    pats[key] = len(pat_list)
                    pat_list.append(tb)
                row.append(("pat", pats[key]))
        btypes.append(row)
    assert len(pat_list) <= MAXPAT, f"too many mask patterns: {len(pat_list)}"
    for row in btypes:
        assert any(t != "skip" for t, _ in row), "fully-masked query tile"
    return btypes, pat_list


def _build_program(btypes, n_pat):
    import concourse.bacc as bacc
    import concourse.tile as tile
    import concourse.mybir as mybir

    dt = mybir.dt
    f32, f32r = dt.float32, dt.float32r
    AF = mybir.ActivationFunctionType

    nc = bacc.Bacc(None, target_bir_lowering=False)

    hT = nc.declare_dram_parameter("hT", [H, S], f32r, isOutput=False)
    wq = nc.declare_dram_parameter("wq", [H, HDPC], f32r, isOutput=False)
    wk = nc.declare_dram_parameter("wk", [H, HDPC], f32r, isOutput=False)
    wv = nc.declare_dram_parameter("wv", [H, HDPC], f32r, isOutput=False)
    wo = nc.declare_dram_parameter("wo", [HDPC, H], f32r, isOutput=False)
    cosq = nc.declare_dram_parameter("cosq", [P, S], f32, isOutput=False)
    sinq = nc.declare_dram_parameter("sinq", [P, S], f32, isOutput=False)
    cosk = nc.declare_dram_parameter("cosk", [P, S], f32, isOutput=False)
    sink = nc.declare_dram_parameter("sink", [P, S], f32, isOutput=False)
    mpat = nc.declare_dram_parameter("mpat", [max(n_pat, 1), P, QBLK], f32,
                                     isOutput=False)
    outp = nc.declare_dram_parameter("outp", [H, S], f32r, isOutput=True)

    NST = S // QBLK  # 4 s-tiles

    with tile.TileContext(nc) as tc:
        with tc.tile_pool(name="res", bufs=1) as res:
            # Q^T, K^T (roped) and V stay resident in SBUF end-to-end:
            # no spill DMA, and exact per-tile deps let attention start
            # as soon as its inputs exist.
            qseg = [[res.tile([P, QBLK], f32r, tag=f"qs_{h}_{st}",
                               name=f"qseg_{h}_{st}")
                     for st in range(NST)] for h in range(NHPC)]
            kseg = [[res.tile([P, QBLK], f32r, tag=f"ks_{h}_{st}",
                               name=f"kseg_{h}_{st}")
                     for st in range(NST)] for h in range(NHPC)]
            vsub = [res.tile([P, HDPC], f32r, tag=f"vs_{i}", name=f"vsub_{i}")
                    for i in range(NSUB)]
            ones_f = res.tile([P, 1], f32, tag="ones_f")
            nc.gpsimd.memset(ones_f[:], 1.0)
            ones = res.tile([P, 1], f32r, tag="ones")
            nc.vector.tensor_copy(ones[:], ones_f[:])

            # ---------------- Phase 1a: Q,K projection + RoPE ---------------
            with tc.tile_pool(name="w1", bufs=1) as w1, \
                 tc.tile_pool(name="tb1", bufs=2) as tb1, \
                 tc.tile_pool(name="hb1", bufs=8) as hb1, \
                 tc.tile_pool(name="tm1", bufs=2) as tm1, \
                 tc.tile_pool(name="ps1", bufs=8, space="PSUM") as ps1:

                wres = {}
                for nm, wdram in (("wq", wq), ("wk", wk)):
                    wt = w1.tile([P, KT * HDPC], f32r, tag=nm, name=nm + "_sb")
                    wtv = wt[:].rearrange("p (k m) -> p k m", k=KT)
                    wsrc = wdram[:].rearrange("(k p) m -> p k m", p=P)
                    for c in range(4):
                        nc.sync.dma_start(wtv[:, c * 4:(c + 1) * 4, :],
                                          wsrc[:, c * 4:(c + 1) * 4, :])
                    wres[nm] = wtv

                def rope_evac(ps, cost, sint, dst):
                    # dst = ps*cos + swap_halves(ps)*sinN  (sign in table)
                    ta = tm1.tile([P, QBLK], f32, tag="ta")
                    tb = tm1.tile([P, QBLK], f32, tag="tb")
                    nc.vector.tensor_mul(ta[:], ps[:], cost[:])
                    nc.vector.tensor_mul(tb[0:64, :], ps[64:128, :],
                                         sint[0:64, :])
                    nc.vector.tensor_mul(tb[64:128, :], ps[0:64, :],
                                         sint[64:128, :])
                    nc.vector.tensor_add(dst[:], ta[:], tb[:])

                for st in range(NST):
                    sc = st * QBLK
                    tabs = {}
                    for nm, src_ in (("cq", cosq), ("sq", sinq),
                                     ("ck", cosk), ("sk", sink)):
                        t = tb1.tile([P, QBLK], f32, tag=nm,
                                     name=f"{nm}_{st}")
                        nc.sync.dma_start(t[:], src_[:, sc:sc + QBLK])
                        tabs[nm] = t
                    hts = []
                    for kt in range(KT):
                        hb = hb1.tile([P, QBLK], f32r, tag="hb", bufs=8,
                                      name=f"hb_{st}_{kt}")
                        nc.sync.dma_start(
                            hb[:], hT[kt * P:(kt + 1) * P, sc:sc + QBLK])
                        hts.append(hb)

                    qk_out = [("wq", h) for h in range(NHPC)] + \
                             [("wk", h) for h in range(NHPC)]
                    pss = [ps1.tile([P, QBLK], f32, tag="ps",
                                    name=f"ps_{st}_{oi}")
                           for oi in range(len(qk_out))]
                    for kt in range(KT):
                        for oi, (nm, h) in enumerate(qk_out):
                            nc.tensor.matmul(
                                pss[oi][:],
                                wres[nm][:, kt, h * HD:(h + 1) * HD],
                                hts[kt][:],
                                start=(kt == 0), stop=(kt == KT - 1))
                    for oi, (nm, h) in enumerate(qk_out):
                        if nm == "wq":
                            rope_evac(pss[oi], tabs["cq"], tabs["sq"],
                                      qseg[h][st])
                        else:
                            rope_evac(pss[oi], tabs["ck"], tabs["sk"],
                                      kseg[h][st])

            # ---------------- Phase 1b: V projection ------------------------
            with tc.tile_pool(name="wv1", bufs=1) as wv1, \
                 tc.tile_pool(name="hv1", bufs=6) as hv1, \
                 tc.tile_pool(name="psV", bufs=8, space="PSUM") as psV:
                wvt = wv1.tile([P, KT * HDPC], f32r, tag="wv", name="wv_sb")
                wvv = wvt[:].rearrange("p (k m) -> p k m", k=KT)
                wvsrc = wv[:].rearrange("(k p) m -> p k m", p=P)
                for c in range(4):
                    nc.sync.dma_start(wvv[:, c * 4:(c + 1) * 4, :],
                                      wvsrc[:, c * 4:(c + 1) * 4, :])
                for st in range(NST):
                    sc = st * QBLK
                    psv = [psV.tile([P, HDPC], f32, tag="pv",
                                    name=f"psv_{st}_{sl}")
                           for sl in range(4)]
                    for kt in range(KT):
                        hv = hv1.tile([P, QBLK], f32r, tag="hv",
                                      name=f"hv_{st}_{kt}")
                        nc.sync.dma_start(
                            hv[:], hT[kt * P:(kt + 1) * P, sc:sc + QBLK])
                        for sl in range(4):
                            nc.tensor.matmul(
                                psv[sl][:],
                                hv[:, sl * P:(sl + 1) * P],
                                wvv[:, kt, :],
                                start=(kt == 0), stop=(kt == KT - 1))
                    for sl in range(4):
                        nc.scalar.copy(vsub[st * 4 + sl][:], psv[sl][:])

            # ---------------- Phase 2: attention ----------------------------
            with tc.tile_pool(name="at2", bufs=1) as at2:
              attn = at2.tile([P, NHPC * S], f32r, tag="attn")
              with tc.tile_pool(name="ex2", bufs=6) as ex2, \
                 tc.tile_pool(name="ms2", bufs=1) as ms2, \
                 tc.tile_pool(name="sm2", bufs=3) as sm2, \
                 tc.tile_pool(name="psS", bufs=3, space="PSUM") as psS, \
                 tc.tile_pool(name="psO", bufs=3, space="PSUM") as psO, \
                 tc.tile_pool(name="psL", bufs=2, space="PSUM") as psL:

                mp = ms2.tile([P, max(n_pat, 1) * QBLK], f32, tag="mp")
                nc.sync.dma_start(
                    mp[:].rearrange("p (n q) -> p n q", q=QBLK),
                    mpat[:].rearrange("n p q -> p n q"))

                for h in range(NHPC):
                    for qt in range(NQT):
                        blocks = [kb for kb in range(NKB)
                                  if btypes[qt][kb][0] != "skip"]
                        po = psO.tile([P, QBLK], f32, tag="po",
                                      name=f"po_{h}_{qt}")
                        pl = psL.tile([1, QBLK], f32, tag="pl",
                                      name=f"pl_{h}_{qt}")
                        for i, kb in enumerate(blocks):
                            first, last = (i == 0), (i == len(blocks) - 1)
                            ps = psS.tile([P, QBLK], f32, tag="ps",
                                          name=f"sc_{h}_{qt}_{kb}")
                            nc.tensor.matmul(
                                ps[:],
                                kseg[h][kb // 4][:, (kb % 4) * KBLK:
                                                 (kb % 4 + 1) * KBLK],
                                qseg[h][qt][:],
                                start=True, stop=True)
                            typ, pid = btypes[qt][kb]
                            if typ == "pat":
                                nc.vector.tensor_add(
                                    ps[:], ps[:],
                                    mp[:, pid * QBLK:(pid + 1) * QBLK])
                            ex = ex2.tile([P, QBLK], f32r, tag="ex")
                            nc.scalar.activation(ex[:], ps[:], AF.Exp)
                            nc.tensor.matmul(
                                po[:], vsub[kb][:, h * HD:(h + 1) * HD],
                                ex[:], start=first, stop=last)
                            nc.tensor.matmul(
                                pl[:], ones[:], ex[:],
                                start=first, stop=last)
                        lr = sm2.tile([1, QBLK], f32, tag="lr")
                        nc.vector.reciprocal_approx_fast(lr[:], pl[:])
                        lb = sm2.tile([P, QBLK], f32, tag="lb")
                        nc.gpsimd.partition_broadcast(lb[:], lr[:])
                        nc.vector.tensor_mul(
                            attn[:, h * S + qt * QBLK:h * S + (qt + 1) * QBLK],
                            po[:], lb[:])

              # ------------ Phase 3: output projection (partial) ------------
              if True:
                with tc.tile_pool(name="wo3", bufs=1) as wo3, \
                     tc.tile_pool(name="ot3", bufs=3) as ot3, \
                     tc.tile_pool(name="psC", bufs=8, space="PSUM") as psC:
                    wos = wo3.tile([P, NHPC * H], f32r, tag="wos")
                    nc.sync.dma_start(
                        wos[:].rearrange("p (k m) -> p k m", k=NHPC),
                        wo[:].rearrange("(k p) m -> p k m", p=P))
                    for mb in range(H // P):
                        pcs = [psC.tile([P, QBLK], f32, tag="pc",
                                        name=f"pc_{mb}_{st3}")
                               for st3 in range(4)]
                        for hk in range(NHPC):
                            for st3 in range(4):
                                nc.tensor.matmul(
                                    pcs[st3][:],
                                    wos[:, hk * H + mb * P:
                                        hk * H + (mb + 1) * P],
                                    attn[:, hk * S + st3 * QBLK:
                                         hk * S + (st3 + 1) * QBLK],
                                    start=(hk == 0), stop=(hk == NHPC - 1))
                        ot = ot3.tile([P, S], f32r, tag="ot")
                        for st3 in range(4):
                            nc.scalar.copy(ot[:, st3 * QBLK:(st3 + 1) * QBLK],
                                           pcs[st3][:])
                        nc.sync.dma_start(outp[mb * P:(mb + 1) * P, :], ot[:])

    nc.finalize()
    return nc


def _get_program(mask2d):
    key = hash(mask2d.tobytes())
    if key not in _prog_cache:
        btypes, pat_list = _classify_mask(mask2d)
        nc = _build_program(btypes, len(pat_list))
        _prog_cache[key] = (nc, btypes, pat_list)
    return _prog_cache[key]


def kernel(hidden_states, rope_cos, rope_sin, attention_mask, w_qkv, w_o):
    from concourse.bass_utils import run_bass_kernel_spmd

    hidden_states = np.asarray(hidden_states, dtype=np.float32)
    rope_cos = np.asarray(rope_cos, dtype=np.float32)
    rope_sin = np.asarray(rope_sin, dtype=np.float32)
    attention_mask = np.asarray(attention_mask, dtype=np.float32)
    w_qkv = np.asarray(w_qkv, dtype=np.float32)
    w_o = np.asarray(w_o, dtype=np.float32)

    mask2d = np.ascontiguousarray(attention_mask.reshape(S, S))
    nc, btypes, pat_list = _get_program(mask2d)
    n_pat = len(pat_list)
    mpat = (np.stack(pat_list) if n_pat
            else np.zeros((1, P, QBLK), np.float32))

    scale = 1.0 / math.sqrt(HD)
    cosT = np.ascontiguousarray(rope_cos.T)          # [HD, S]
    sinT = rope_sin.T.copy()
    # fold the rotate-half sign into the table: out = x*cos + swap(x)*sinN
    sinT[0:64, :] *= -1.0
    sinT = np.ascontiguousarray(sinT)
    cosq = np.ascontiguousarray(cosT * scale)
    sinq = np.ascontiguousarray(sinT * scale)

    hTs = [np.ascontiguousarray(hidden_states[b].T) for b in range(B)]

    in_maps = []
    for c in range(8):
        b, hg = c // 4, c % 4
        r0 = hg * HDPC
        in_maps.append({
            "hT": hTs[b],
            "wq": np.ascontiguousarray(w_qkv[r0:r0 + HDPC, :].T),
            "wk": np.ascontiguousarray(w_qkv[H + r0:H + r0 + HDPC, :].T),
            "wv": np.ascontiguousarray(w_qkv[2 * H + r0:2 * H + r0 + HDPC, :].T),
            "wo": np.ascontiguousarray(w_o[:, r0:r0 + HDPC].T),
            "cosq": cosq, "sinq": sinq, "cosk": cosT, "sink": sinT,
            "mpat": mpat,
        })

    import os
    kw = {}
    if os.environ.get("BASS_KERNEL_TRACE"):
        kw["trace"] = True
    res = run_bass_kernel_spmd(nc, in_maps, list(range(8)), **kw)
    global LAST_RESULTS
    LAST_RESULTS = res

    out = np.empty((B, S, H), dtype=np.float32)
    for b in range(B):
        acc = np.zeros((H, S), dtype=np.float64)
        for hg in range(4):
            acc += res.results[b * 4 + hg]["outp"].astype(np.float64)
        out[b] = acc.T.astype(np.float32)
    return out
